# revision 14
# baseline (speedup 1.0000x reference)
"""Trainium2 Bass kernel for nn_ChiStream: 3 chained cross-attention stages
between two token streams + per-stream 1x1-conv/ReLU/BN output heads.

Sharding: 8 cores = 4 batches x 2 head-groups (4 heads / 256 channels each).
Within a stage each core computes its head-group's attention deltas for both
streams; a pair-wise AllGather exchanges deltas so every core holds the full
updated residual streams for the next stage.

Layouts: residual streams are channel-major [512, 1568] in SBUF (4 subtiles
of 128 partitions).  Scores are computed transposed (k-tokens on partitions)
so softmax normalization uses a K=1 ones-matmul broadcast; the AV matmul
contracts k-tokens directly.  All matmuls run as float32r (full-rate fp32).

PSUM budget (8 banks, statically pooled): mm 3 + scores 2 + out-acc 2 + rb 1.
"""

import sys
from contextlib import ExitStack

import numpy as np

sys.path.insert(0, "/opt/trn_rl_repo")

import concourse.bass as bass  # noqa: E402,F401
import concourse.mybir as mybir  # noqa: E402
import concourse.tile as tile  # noqa: E402
from concourse import bacc  # noqa: E402
from concourse.bass_utils import run_bass_kernel_spmd  # noqa: E402

F32 = mybir.dt.float32
F32R = mybir.dt.float32r
BF16 = mybir.dt.bfloat16
AF = mybir.ActivationFunctionType
ALU = mybir.AluOpType

N_CORES = 8
N = 1568            # tokens per stream
C = 512             # channels
CSUB = 4            # channel subtiles of 128
P = 128
NQ = 392            # query-chunk size (4 chunks, each fits a PSUM bank)
NCHUNK = 4
KT_SIZES = [128] * 12 + [32]   # key tiles (1568 = 12*128 + 32)
NKT = len(KT_SIZES)
NPAD = NKT * 128               # key tokens padded to 1664 (13 full tiles)
HLOC = 4            # heads per core
HD = 64             # head dim
STAGES = 3
EPS = 1e-5
REPLICA_GROUPS = [[0, 1], [2, 3], [4, 5], [6, 7]]


def _r(ap):
    """Matmul operands are stored as float32r natively."""
    return ap


def _chunk(ap, c):
    return ap[..., c * NQ:(c + 1) * NQ]


def _layernorm(nc, ps, sb, src_q, g_row, beta_col, cons):
    """Channel-dim LayerNorm of channel-major src_q -> ln tile (gamma folded
    into the per-token broadcast matmuls, beta added per-partition)."""
    ones_sb, eps_sb = cons
    sq_sb = sb["big"].tile([P, CSUB, N], F32R, tag="big", name="sq")
    nc.vector.tensor_mul(sq_sb[:], src_q[:], src_q[:])

    s1_sb = sb["sm"].tile([1, NCHUNK, NQ], F32R, tag="s1", name="s1")
    s2_sb = sb["sm"].tile([1, NCHUNK, NQ], F32R, tag="s2", name="s2")
    a_sb = sb["sm"].tile([1, NCHUNK, NQ], F32R, tag="a", name="a")
    for c in range(NCHUNK):
        s1_ps = ps["mm"].tile([1, NQ], F32, tag="mm", name="s1ps")
        for su in range(CSUB):
            nc.tensor.matmul(
                s1_ps[:], _r(ones_sb[:, 0:1]), _r(_chunk(src_q[:, su], c)),
                start=(su == 0), stop=(su == CSUB - 1))
        nc.vector.tensor_copy(s1_sb[0:1, c], s1_ps[:])
        s2_ps = ps["mm"].tile([1, NQ], F32, tag="mm", name="s2ps")
        for su in range(CSUB):
            nc.tensor.matmul(
                s2_ps[:], _r(ones_sb[:, 0:1]), _r(_chunk(sq_sb[:, su], c)),
                start=(su == 0), stop=(su == CSUB - 1))
        nc.vector.tensor_copy(s2_sb[0:1, c], s2_ps[:])

    # s1 -> mean -> nb = -mean*a ; s2 -> var -> ln(var+eps) ; a = rsqrt
    nc.vector.tensor_scalar_mul(s1_sb[:], s1_sb[:], 1.0 / C)          # mean
    nc.vector.tensor_mul(a_sb[:], s1_sb[:], s1_sb[:])                 # mean^2
    nc.vector.scalar_tensor_tensor(
        s2_sb[:], s2_sb[:], 1.0 / C, a_sb[:], ALU.mult, ALU.subtract)  # var
    nc.scalar.activation(s2_sb[:], s2_sb[:], AF.Ln, bias=eps_sb[:])
    nc.scalar.activation(a_sb[:], s2_sb[:], AF.Exp, scale=-0.5)       # rsqrt
    nc.vector.scalar_tensor_tensor(
        s1_sb[:], s1_sb[:], -1.0, a_sb[:], ALU.mult, ALU.mult)        # nb

    # ln = t * (gamma (x) a) + (gamma (x) nb) + beta
    ln_sb = sb["big"].tile([P, CSUB, N], F32R, tag="big", name="ln")
    for su in range(CSUB):
        for c in range(NCHUNK):
            a_ps = ps["mm"].tile([P, NQ], F32, tag="mm", name="aps")
            b_ps = ps["mm"].tile([P, NQ], F32, tag="mm", name="bps")
            nc.tensor.matmul(a_ps[:], _r(g_row[0:1, su]), _r(a_sb[0:1, c]),
                             start=True, stop=True)
            nc.tensor.matmul(b_ps[:], _r(g_row[0:1, su]), _r(s1_sb[0:1, c]),
                             start=True, stop=True)
            dst = _chunk(ln_sb[:, su], c)
            nc.vector.tensor_mul(dst, _chunk(src_q[:, su], c), a_ps[:])
            nc.vector.scalar_tensor_tensor(
                dst, dst, beta_col[:, su:su + 1], b_ps[:], ALU.add, ALU.add)
    return ln_sb


def _proj_cm(nc, ps, src, w_sb, b_sb, dst):
    """Channel-major projection: dst[., m, n] = (W.T slice).T @ src + bias."""
    for m in range(2):
        for c in range(NCHUNK):
            pr_ps = ps["mm"].tile([P, NQ], F32, tag="mm", name="proj")
            for su in range(CSUB):
                nc.tensor.matmul(
                    pr_ps[:], _r(w_sb[:, su, m * P:(m + 1) * P]),
                    _r(_chunk(src[:, su], c)),
                    start=(su == 0), stop=(su == CSUB - 1))
            nc.vector.tensor_scalar_add(
                _chunk(dst[:, m], c), pr_ps[:], b_sb[:, m:m + 1])


def _proj_k(nc, ps, src, w_sb, b_sb, k_sb):
    """K projection into per-head 128-partition slots: even heads occupy
    partitions 0:64 of their slot, odd heads 64:128; the other half stays
    zero (set once at kernel start) so scores contract over K=128."""
    for m in range(2):
        for c in range(NCHUNK):
            pr_ps = ps["mm"].tile([P, NQ], F32, tag="mm", name="kproj")
            for su in range(CSUB):
                nc.tensor.matmul(
                    pr_ps[:], _r(w_sb[:, su, m * P:(m + 1) * P]),
                    _r(_chunk(src[:, su], c)),
                    start=(su == 0), stop=(su == CSUB - 1))
            nc.vector.tensor_scalar_add(
                k_sb[0:HD, 2 * m, c * NQ:(c + 1) * NQ],
                pr_ps[0:HD], b_sb[0:HD, m:m + 1])
            nc.vector.tensor_scalar_add(
                k_sb[HD:P, 2 * m + 1, c * NQ:(c + 1) * NQ],
                pr_ps[HD:P], b_sb[HD:P, m:m + 1])


def _proj_v(nc, ps, sb, src_kv, wv_sb, ones_sb):
    """Token-major V with a trailing ones column: [token, kt, head, 65]."""
    v_sb = sb["v"].tile([P, NKT, HLOC, HD + 1], BF16, tag="v", name="v")
    nc.vector.tensor_copy(v_sb[:, :, :, HD:HD + 1],
                          ones_sb[:, 0:1].to_broadcast((P, NKT, HLOC, 1)))
    # zero the invalid token rows of the last key tile (v AND its ones col);
    # split on the 32/64 partition-group boundaries the ISA requires
    nc.vector.tensor_scalar_mul(
        v_sb[32:64, NKT - 1],
        ones_sb[32:64, 0:1].to_broadcast((32, HLOC, HD + 1)), 0.0)
    nc.vector.tensor_scalar_mul(
        v_sb[64:P, NKT - 1],
        ones_sb[64:P, 0:1].to_broadcast((HD, HLOC, HD + 1)), 0.0)
    for kt in range(NKT):
        rows = KT_SIZES[kt]
        vp_ps = ps["mm"].tile([P, 256], F32, tag="mm", name="vproj")
        for su in range(CSUB):
            nc.tensor.matmul(
                vp_ps[:rows], _r(src_kv[:, su, kt * P:kt * P + rows]),
                _r(wv_sb[:, su]),
                start=(su == 0), stop=(su == CSUB - 1))
        nc.vector.tensor_copy(
            v_sb[:rows, kt, :, 0:HD],
            vp_ps[:rows].rearrange("p (h d) -> p h d", h=HLOC))
    return v_sb


def _attention(nc, ps, sb, q_sb, k_sb, v_sb, cc_in_s, ones_sb):
    """Flash-style attention, K=128 full-array matmuls throughout.

    k_sb[:, h] holds head h's keys in the partition half matching where the
    packed q tile keeps that head's queries, zeros in the other half - so a
    128-deep contraction computes exactly the 64-dim head dot product.
    Exp is batched over kt-pairs; softmax normalization is deferred to a
    per-head batch epilogue (one strided reciprocal)."""
    KT_PAIRS = [(0, 1), (2, 3), (4, 5), (6, 7), (8, 9), (10, 11), (12,)]
    for h in range(HLOC):
        dh_sb = sb["dn"].tile([HD + 1, NCHUNK, NQ], F32R, tag="dn", name="dn")
        for c in range(NCHUNK):
            oe_ps = ps["oe"].tile([HD + 1, NQ], F32, tag="oe", name="oe")
            for pair in KT_PAIRS:
                sc_ps = ps["sc"].tile([P, 2, 512], F32, tag="sc", name="sc")
                for j, kt in enumerate(pair):
                    nc.tensor.matmul(
                        sc_ps[:, j, :NQ], _r(k_sb[:, h, kt * P:(kt + 1) * P]),
                        _r(_chunk(q_sb[:, h // 2], c)), start=True, stop=True)
                p_sb = sb["p"].tile([P, 2, NQ], BF16, tag="p", name="p")
                npair = len(pair)
                nc.scalar.activation(
                    p_sb[:, :npair], sc_ps[:, :npair, :NQ], AF.Exp,
                    scale=float(HD) ** -0.5)
                for j, kt in enumerate(pair):
                    nc.tensor.matmul(
                        oe_ps[:], _r(v_sb[:, kt, h]), _r(p_sb[:, j]),
                        start=(kt == 0), stop=(kt == NKT - 1))
            nc.vector.tensor_copy(dh_sb[:, c], oe_ps[:])
        # batch-normalize: one in-place reciprocal over all 4 chunk rowsums
        with nc.allow_low_precision(reason="softmax denom recip"):
            nc.vector.reciprocal(dh_sb[HD:HD + 1], dh_sb[HD:HD + 1])
        for c in range(NCHUNK):
            rb_ps = ps["mm"].tile([HD, NQ], F32, tag="mm", name="rb")
            nc.tensor.matmul(rb_ps[:], _r(ones_sb[HD:HD + 1, 0:HD]),
                             _r(dh_sb[HD:HD + 1, c]),
                             start=True, stop=True)
            ds_sb = sb["ds"].tile([HD, NQ], F32, tag="ds", name="ds")
            nc.vector.tensor_mul(ds_sb[:], dh_sb[0:HD, c], rb_ps[:])
            nc.sync.dma_start(
                cc_in_s[HD * h:HD * (h + 1), c * NQ:(c + 1) * NQ], ds_sb[:])


def _stage_stream(nc, ps, sb, ins, i, s, t_sb, kpad_sb, cc_in_s, cons):
    src_q = t_sb[s]
    src_kv = t_sb[1 - s]

    wq_sb = sb["w"].tile([P, CSUB, 256], F32R, tag="wq", name="wq")
    wk_sb = sb["w"].tile([P, CSUB, 256], F32R, tag="wk", name="wk")
    wv_sb = sb["w"].tile([P, CSUB, 256], F32R, tag="wv", name="wv")
    nc.sync.dma_start(wq_sb[:], ins["wq"][i, s].rearrange("(su p) m -> p su m", p=P))
    nc.sync.dma_start(wk_sb[:], ins["wk"][i, s].rearrange("(su p) m -> p su m", p=P))
    nc.sync.dma_start(wv_sb[:], ins["wv"][i, s].rearrange("(su p) m -> p su m", p=P))
    g_row = sb["cst"].tile([1, CSUB, P], F32R, tag="lng", name="lng")
    nc.sync.dma_start(g_row[:],
                      ins["lng"][i, s].rearrange("(su p) -> su p", p=P)[None])
    beta_col = sb["cst"].tile([P, CSUB], F32, tag="lnb", name="lnb")
    nc.sync.dma_start(beta_col[:], ins["lnb"][i, s].rearrange("(su p) -> p su", p=P))
    bq_sb = sb["cst"].tile([P, 2], F32, tag="bq", name="bq")
    bk_sb = sb["cst"].tile([P, 2], F32, tag="bk", name="bk")
    nc.sync.dma_start(bq_sb[:], ins["bq"][i, s].rearrange("(m p) -> p m", p=P))
    nc.sync.dma_start(bk_sb[:], ins["bk"][i, s].rearrange("(m p) -> p m", p=P))

    ln_sb = _layernorm(nc, ps, sb, src_q, g_row, beta_col, cons)

    q_sb = sb["qk"].tile([P, 2, N], BF16, tag="q", name="q")
    k_sb = kpad_sb
    _proj_cm(nc, ps, ln_sb, wq_sb, bq_sb, q_sb)
    _proj_k(nc, ps, src_kv, wk_sb, bk_sb, k_sb)
    v_sb = _proj_v(nc, ps, sb, src_kv, wv_sb, cons[0])

    _attention(nc, ps, sb, q_sb, k_sb, v_sb, cc_in_s, cons[0])


def _residual_update(nc, sb, ins, i, s, t_sb, cc_out):  # cc_out: [2,256,N]
    bv_sb = sb["cst"].tile([P, CSUB], F32, tag="bv", name="bv")
    nc.sync.dma_start(bv_sb[:], ins["bv"][i, s].rearrange("(su p) -> p su", p=P))
    for r in range(2):
        g_sb = sb["big"].tile([P, 2, N], F32, tag="big", name="gather")
        nc.sync.dma_start(g_sb[:], cc_out[r].rearrange("(m p) n -> p m n", p=P))
        for m in range(2):
            su = 2 * r + m
            nc.vector.scalar_tensor_tensor(
                t_sb[s][:, su], g_sb[:, m], bv_sb[:, su:su + 1],
                t_sb[s][:, su], ALU.add, ALU.add)


def _out_head(nc, ps, sb, ins, s, t_sb, out):
    wo_sb = sb["w"].tile([P, CSUB, 256], F32R, tag="wq", name="wo")
    nc.sync.dma_start(wo_sb[:], ins["wo"][s].rearrange("(su p) m -> p su m", p=P))
    ob_sb = sb["cst"].tile([P, 2], F32, tag="ob", name="ob")
    bns_sb = sb["cst"].tile([P, 2], F32, tag="bns", name="bns")
    bnb_sb = sb["cst"].tile([P, 2], F32, tag="bnb", name="bnb")
    nc.sync.dma_start(ob_sb[:], ins["ob"][s].rearrange("(m p) -> p m", p=P))
    nc.sync.dma_start(bns_sb[:], ins["bns"][s].rearrange("(m p) -> p m", p=P))
    nc.sync.dma_start(bnb_sb[:], ins["bnb"][s].rearrange("(m p) -> p m", p=P))
    o_sb = sb["qk"].tile([P, 2, N], F32, tag="q", name="ohead")
    for m in range(2):
        for c in range(NCHUNK):
            z_ps = ps["mm"].tile([P, NQ], F32, tag="mm", name="zps")
            for su in range(CSUB):
                nc.tensor.matmul(
                    z_ps[:], _r(wo_sb[:, su, m * P:(m + 1) * P]),
                    _r(_chunk(t_sb[s][:, su], c)),
                    start=(su == 0), stop=(su == CSUB - 1))
            dst = _chunk(o_sb[:, m], c)
            nc.scalar.activation(dst, z_ps[:], AF.Relu, bias=ob_sb[:, m:m + 1])
            nc.vector.tensor_scalar(
                dst, dst, bns_sb[:, m:m + 1], bnb_sb[:, m:m + 1], ALU.mult, ALU.add)
    nc.sync.dma_start(out[s].rearrange("(m p) n -> p m n", p=P), o_sb[:])


def build_nc():
    nc = bacc.Bacc("TRN2", target_bir_lowering=False, debug=False,
                   num_devices=N_CORES)

    ins = {}
    ins["x"] = nc.dram_tensor("x", [2, C, N], F32R, kind="ExternalInput")
    ins["ones"] = nc.dram_tensor("ones", [P, P], F32R, kind="ExternalInput")
    ins["wq"] = nc.dram_tensor("wq", [STAGES, 2, C, 256], F32R, kind="ExternalInput")
    ins["wk"] = nc.dram_tensor("wk", [STAGES, 2, C, 256], F32R, kind="ExternalInput")
    ins["wv"] = nc.dram_tensor("wv", [STAGES, 2, C, 256], F32R, kind="ExternalInput")
    ins["bq"] = nc.dram_tensor("bq", [STAGES, 2, 256], F32, kind="ExternalInput")
    ins["bk"] = nc.dram_tensor("bk", [STAGES, 2, 256], F32, kind="ExternalInput")
    ins["bv"] = nc.dram_tensor("bv", [STAGES, 2, C], F32, kind="ExternalInput")
    ins["lng"] = nc.dram_tensor("lng", [STAGES, 2, C], F32R, kind="ExternalInput")
    ins["lnb"] = nc.dram_tensor("lnb", [STAGES, 2, C], F32, kind="ExternalInput")
    ins["wo"] = nc.dram_tensor("wo", [2, C, 256], F32R, kind="ExternalInput")
    ins["ob"] = nc.dram_tensor("ob", [2, 256], F32, kind="ExternalInput")
    ins["bns"] = nc.dram_tensor("bns", [2, 256], F32, kind="ExternalInput")
    ins["bnb"] = nc.dram_tensor("bnb", [2, 256], F32, kind="ExternalInput")
    out = nc.dram_tensor("out", [2, 256, N], F32, kind="ExternalOutput")

    with ExitStack() as ctx:
        tc = ctx.enter_context(tile.TileContext(nc))
        sb = {
            "res": ctx.enter_context(tc.tile_pool(name="res", bufs=1)),
            "big": ctx.enter_context(tc.tile_pool(name="big", bufs=1)),
            "qk": ctx.enter_context(tc.tile_pool(name="qk", bufs=1)),
            "v": ctx.enter_context(tc.tile_pool(name="vv", bufs=1)),
            "w": ctx.enter_context(tc.tile_pool(name="wt", bufs=1)),
            "sm": ctx.enter_context(tc.tile_pool(name="sm", bufs=1)),
            "cst": ctx.enter_context(tc.tile_pool(name="cst", bufs=2)),
            "p": ctx.enter_context(tc.tile_pool(name="pp", bufs=3)),
            "dn": ctx.enter_context(tc.tile_pool(name="dn", bufs=1)),
            "ds": ctx.enter_context(tc.tile_pool(name="dsp", bufs=4)),
        }
        ps = {
            "mm": ctx.enter_context(tc.tile_pool(name="ps_mm", bufs=2, space="PSUM")),
            "sc": ctx.enter_context(tc.tile_pool(name="ps_sc", bufs=2, space="PSUM")),
            "oe": ctx.enter_context(tc.tile_pool(name="ps_oe", bufs=2, space="PSUM")),
        }
        dram = ctx.enter_context(tc.tile_pool(name="dram", bufs=1, space="DRAM"))

        t_sb = [sb["res"].tile([P, CSUB, N], F32R, tag=f"t{s}", name=f"t{s}")
                for s in (0, 1)]
        kpad_sb = sb["res"].tile([P, HLOC, NPAD], BF16, tag="kpad", name="kpad")
        ones_sb = sb["res"].tile([P, P], F32R, tag="ones", name="ones")
        nc.sync.dma_start(ones_sb[:], ins["ones"][:])
        eps_sb = sb["res"].tile([1, 1], F32, tag="eps", name="eps")
        nc.vector.memset(eps_sb[:], EPS)
        cons = (ones_sb, eps_sb)
        for s in (0, 1):
            nc.sync.dma_start(t_sb[s][:],
                              ins["x"][s].rearrange("(su p) n -> p su n", p=P))
        # zero the pad halves (even-head slots: rows 64:128, odd: 0:64) and
        # the padded key-token columns; written once, preserved across stages
        zsrc = ones_sb[:, 0:1]
        nc.vector.tensor_scalar_mul(
            kpad_sb[HD:P, 0:HLOC:2],
            zsrc[HD:P].to_broadcast((HD, 2, NPAD)), 0.0)
        nc.vector.tensor_scalar_mul(
            kpad_sb[0:HD, 1:HLOC:2],
            zsrc[0:HD].to_broadcast((HD, 2, NPAD)), 0.0)
        nc.vector.tensor_scalar_mul(
            kpad_sb[0:HD, 0:HLOC:2, N:NPAD],
            zsrc[0:HD].to_broadcast((HD, 2, NPAD - N)), 0.0)
        nc.vector.tensor_scalar_mul(
            kpad_sb[HD:P, 1:HLOC:2, N:NPAD],
            zsrc[HD:P].to_broadcast((HD, 2, NPAD - N)), 0.0)

        for i in range(STAGES):
            cc_outs = []
            for s in (0, 1):
                cc_in = dram.tile([256, N], F32, name="cc_in")
                cc_out = dram.tile([2, 256, N], F32, name="cc_out")
                cc_outs.append(cc_out)
                _stage_stream(nc, ps, sb, ins, i, s, t_sb, kpad_sb, cc_in, cons)
                nc.gpsimd.collective_compute(
                    "AllGather", ALU.bypass, replica_groups=REPLICA_GROUPS,
                    ins=[cc_in[:]], outs=[cc_out[:]])
            for s in (0, 1):
                _residual_update(nc, sb, ins, i, s, t_sb, cc_outs[s])

        for s in (0, 1):
            _out_head(nc, ps, sb, ins, s, t_sb, out)

    nc.compile()
    return nc


_NC_CACHE = {}


def _get_nc():
    if "nc" not in _NC_CACHE:
        _NC_CACHE["nc"] = build_nc()
    return _NC_CACHE["nc"]


def _prep_in_maps(inputs):
    f = lambda k: np.ascontiguousarray(np.asarray(inputs[k], np.float32))
    x1, x2 = f("x1"), f("x2")
    bn_fold = np.float32(1.0 / np.sqrt(np.float32(1.0) + np.float32(EPS)))

    per_g = []
    for g in range(2):
        gs = slice(256 * g, 256 * (g + 1))
        d = {
            "wq": np.stack([np.stack([f("Wq1")[i].T[:, gs], f("Wq2")[i].T[:, gs]])
                            for i in range(STAGES)]),
            "wk": np.stack([np.stack([f("Wk1")[i].T[:, gs], f("Wk2")[i].T[:, gs]])
                            for i in range(STAGES)]),
            "wv": np.stack([np.stack([f("Wv1")[i].T[:, gs], f("Wv2")[i].T[:, gs]])
                            for i in range(STAGES)]),
            "bq": np.stack([np.stack([f("bq1")[i][gs], f("bq2")[i][gs]])
                            for i in range(STAGES)]),
            "bk": np.stack([np.stack([f("bk1")[i][gs], f("bk2")[i][gs]])
                            for i in range(STAGES)]),
            "bv": np.stack([np.stack([f("bv1")[i], f("bv2")[i]])
                            for i in range(STAGES)]),
            "lng": np.stack([np.stack([f("ln1_g")[i], f("ln2_g")[i]])
                             for i in range(STAGES)]),
            "lnb": np.stack([np.stack([f("ln1_b")[i], f("ln2_b")[i]])
                             for i in range(STAGES)]),
            "wo": np.stack([f("out1_w").T[:, gs], f("out2_w").T[:, gs]]),
            "ob": np.stack([f("out1_b")[gs], f("out2_b")[gs]]),
            "bns": np.stack([f("bn1_g")[gs] * bn_fold, f("bn2_g")[gs] * bn_fold]),
            "bnb": np.stack([f("bn1_b")[gs], f("bn2_b")[gs]]),
        }
        per_g.append({k: np.ascontiguousarray(v) for k, v in d.items()})

    in_maps = []
    for core in range(N_CORES):
        b, g = core // 2, core % 2
        m = dict(per_g[g])
        m["x"] = np.ascontiguousarray(
            np.stack([x1[b].reshape(C, N), x2[b].reshape(C, N)]))
        m["ones"] = np.ones((P, P), np.float32)
        in_maps.append(m)
    return in_maps


def kernel(**inputs):
    nc = _get_nc()
    in_maps = _prep_in_maps(inputs)
    res = run_bass_kernel_spmd(nc, in_maps, core_ids=list(range(N_CORES)))
    z1 = np.empty((4, C, 8, 14, 14), np.float32)
    z2 = np.empty((4, C, 32, 7, 7), np.float32)
    for core in range(N_CORES):
        b, g = core // 2, core % 2
        o = res.results[core]["out"]
        z1[b, 256 * g:256 * (g + 1)] = o[0].reshape(256, 8, 14, 14)
        z2[b, 256 * g:256 * (g + 1)] = o[1].reshape(256, 32, 7, 7)
    return (z1, z2)


# revision 16
# speedup vs baseline: 3129.2911x; 3129.2911x over previous
"""Trainium2 Bass kernel for nn_ChiStream: 3 chained cross-attention stages
between two token streams + per-stream 1x1-conv/ReLU/BN output heads.

Sharding: 8 cores = 4 batches x 2 head-groups (4 heads / 256 channels each).
Within a stage each core computes its head-group's attention deltas for both
streams; a pair-wise AllGather exchanges deltas so every core holds the full
updated residual streams for the next stage.

Layouts: residual streams are channel-major [512, 1568] in SBUF (4 subtiles
of 128 partitions).  Scores are computed transposed (k-tokens on partitions)
so softmax normalization uses a K=1 ones-matmul broadcast; the AV matmul
contracts k-tokens directly.  All matmuls run as float32r (full-rate fp32).

PSUM budget (8 banks, statically pooled): mm 3 + scores 2 + out-acc 2 + rb 1.
"""

import sys
from contextlib import ExitStack

import numpy as np

sys.path.insert(0, "/opt/trn_rl_repo")

import concourse.bass as bass  # noqa: E402,F401
import concourse.mybir as mybir  # noqa: E402
import concourse.tile as tile  # noqa: E402
from concourse import bacc  # noqa: E402
from concourse.bass_utils import run_bass_kernel_spmd  # noqa: E402

F32 = mybir.dt.float32
F32R = mybir.dt.float32r
BF16 = mybir.dt.bfloat16
AF = mybir.ActivationFunctionType
ALU = mybir.AluOpType

N_CORES = 8
N = 1568            # tokens per stream
C = 512             # channels
CSUB = 4            # channel subtiles of 128
P = 128
NQ = 392            # query-chunk size (4 chunks, each fits a PSUM bank)
NCHUNK = 4
KT_SIZES = [128] * 12 + [32]   # key tiles (1568 = 12*128 + 32)
NKT = len(KT_SIZES)
NPAD = NKT * 128               # key tokens padded to 1664 (13 full tiles)
HLOC = 4            # heads per core
HD = 64             # head dim
STAGES = 3
EPS = 1e-5
REPLICA_GROUPS = [[0, 1], [2, 3], [4, 5], [6, 7]]


def _r(ap):
    """Matmul operands are stored as float32r natively."""
    return ap


def _chunk(ap, c):
    return ap[..., c * NQ:(c + 1) * NQ]


def _layernorm(nc, ps, sb, src_q, src_qb, g_row, beta_col, cons):
    """Channel-dim LayerNorm of channel-major src_q -> ln tile (gamma folded
    into the per-token broadcast matmuls, beta added per-partition)."""
    ones_sb, eps_sb, ones_bf = cons
    sq_sb = sb["bigb"].tile([P, CSUB, N], BF16, tag="bigb", name="sq")
    nc.vector.tensor_mul(sq_sb[:], src_qb[:], src_qb[:])

    s1_sb = sb["sm"].tile([1, NCHUNK, NQ], F32R, tag="s1", name="s1")
    s2_sb = sb["sm"].tile([1, NCHUNK, NQ], F32R, tag="s2", name="s2")
    a_sb = sb["sm"].tile([1, NCHUNK, NQ], F32R, tag="a", name="a")
    for c in range(NCHUNK):
        s1_ps = ps["mm"].tile([1, NQ], F32, tag="mm", name="s1ps")
        for su in range(CSUB):
            nc.tensor.matmul(
                s1_ps[:], _r(ones_bf[:, 0:1]), _r(_chunk(src_qb[:, su], c)),
                start=(su == 0), stop=(su == CSUB - 1))
        nc.vector.tensor_copy(s1_sb[0:1, c], s1_ps[:])
        s2_ps = ps["mm"].tile([1, NQ], F32, tag="mm", name="s2ps")
        for su in range(CSUB):
            nc.tensor.matmul(
                s2_ps[:], _r(ones_bf[:, 0:1]), _r(_chunk(sq_sb[:, su], c)),
                start=(su == 0), stop=(su == CSUB - 1))
        nc.vector.tensor_copy(s2_sb[0:1, c], s2_ps[:])

    # s1 -> mean -> nb = -mean*a ; s2 -> var -> ln(var+eps) ; a = rsqrt
    nc.vector.tensor_scalar_mul(s1_sb[:], s1_sb[:], 1.0 / C)          # mean
    nc.vector.tensor_mul(a_sb[:], s1_sb[:], s1_sb[:])                 # mean^2
    nc.vector.scalar_tensor_tensor(
        s2_sb[:], s2_sb[:], 1.0 / C, a_sb[:], ALU.mult, ALU.subtract)  # var
    nc.scalar.activation(s2_sb[:], s2_sb[:], AF.Ln, bias=eps_sb[:])
    nc.scalar.activation(a_sb[:], s2_sb[:], AF.Exp, scale=-0.5)       # rsqrt
    nc.vector.scalar_tensor_tensor(
        s1_sb[:], s1_sb[:], -1.0, a_sb[:], ALU.mult, ALU.mult)        # nb

    # ln = t * (gamma (x) a) + (gamma (x) nb) + beta
    ln_sb = sb["bigb"].tile([P, CSUB, N], BF16, tag="bigb", name="ln")
    for su in range(CSUB):
        for c in range(NCHUNK):
            a_ps = ps["mm"].tile([P, NQ], F32, tag="mm", name="aps")
            b_ps = ps["mm"].tile([P, NQ], F32, tag="mm", name="bps")
            nc.tensor.matmul(a_ps[:], _r(g_row[0:1, su]), _r(a_sb[0:1, c]),
                             start=True, stop=True)
            nc.tensor.matmul(b_ps[:], _r(g_row[0:1, su]), _r(s1_sb[0:1, c]),
                             start=True, stop=True)
            dst = _chunk(ln_sb[:, su], c)
            nc.vector.tensor_mul(dst, _chunk(src_q[:, su], c), a_ps[:])
            nc.vector.scalar_tensor_tensor(
                dst, dst, beta_col[:, su:su + 1], b_ps[:], ALU.add, ALU.add)
    return ln_sb


def _proj_cm(nc, ps, src, w_sb, b_sb, dst):
    """Channel-major projection: dst[., m, n] = (W.T slice).T @ src + bias."""
    for m in range(2):
        for c in range(NCHUNK):
            pr_ps = ps["mm"].tile([P, NQ], F32, tag="mm", name="proj")
            for su in range(CSUB):
                nc.tensor.matmul(
                    pr_ps[:], _r(w_sb[:, su, m * P:(m + 1) * P]),
                    _r(_chunk(src[:, su], c)),
                    start=(su == 0), stop=(su == CSUB - 1))
            nc.vector.tensor_scalar_add(
                _chunk(dst[:, m], c), pr_ps[:], b_sb[:, m:m + 1])


def _proj_k(nc, ps, src, w_sb, b_sb, k_sb):
    """K projection into per-head 128-partition slots: even heads occupy
    partitions 0:64 of their slot, odd heads 64:128; the other half stays
    zero (set once at kernel start) so scores contract over K=128."""
    for m in range(2):
        for c in range(NCHUNK):
            pr_ps = ps["mm"].tile([P, NQ], F32, tag="mm", name="kproj")
            for su in range(CSUB):
                nc.tensor.matmul(
                    pr_ps[:], _r(w_sb[:, su, m * P:(m + 1) * P]),
                    _r(_chunk(src[:, su], c)),
                    start=(su == 0), stop=(su == CSUB - 1))
            nc.vector.tensor_scalar_add(
                k_sb[0:HD, 2 * m, c * NQ:(c + 1) * NQ],
                pr_ps[0:HD], b_sb[0:HD, m:m + 1])
            nc.vector.tensor_scalar_add(
                k_sb[HD:P, 2 * m + 1, c * NQ:(c + 1) * NQ],
                pr_ps[HD:P], b_sb[HD:P, m:m + 1])


def _proj_v(nc, ps, sb, src_kv, wv_sb, ones_sb):
    """Token-major V with a trailing ones column: [token, kt, head, 65]."""
    v_sb = sb["v"].tile([P, NKT, HLOC, HD + 1], BF16, tag="v", name="v")
    nc.vector.tensor_copy(v_sb[:, :, :, HD:HD + 1],
                          ones_sb[:, 0:1].to_broadcast((P, NKT, HLOC, 1)))
    # zero the invalid token rows of the last key tile (v AND its ones col);
    # split on the 32/64 partition-group boundaries the ISA requires
    nc.vector.tensor_scalar_mul(
        v_sb[32:64, NKT - 1],
        ones_sb[32:64, 0:1].to_broadcast((32, HLOC, HD + 1)), 0.0)
    nc.vector.tensor_scalar_mul(
        v_sb[64:P, NKT - 1],
        ones_sb[64:P, 0:1].to_broadcast((HD, HLOC, HD + 1)), 0.0)
    for kt in range(NKT):
        rows = KT_SIZES[kt]
        vp_ps = ps["mm"].tile([P, 256], F32, tag="mm", name="vproj")
        for su in range(CSUB):
            nc.tensor.matmul(
                vp_ps[:rows], _r(src_kv[:, su, kt * P:kt * P + rows]),
                _r(wv_sb[:, su]),
                start=(su == 0), stop=(su == CSUB - 1))
        nc.vector.tensor_copy(
            v_sb[:rows, kt, :, 0:HD],
            vp_ps[:rows].rearrange("p (h d) -> p h d", h=HLOC))
    return v_sb


def _attention(nc, ps, sb, q_sb, k_sb, v_sb, cc_in_s, ones_sb):
    """Flash-style attention, K=128 full-array matmuls throughout.

    k_sb[:, h] holds head h's keys in the partition half matching where the
    packed q tile keeps that head's queries, zeros in the other half - so a
    128-deep contraction computes exactly the 64-dim head dot product.
    Exp is batched over kt-pairs; softmax normalization is deferred to a
    per-head batch epilogue (one strided reciprocal)."""
    KT_PAIRS = [(0, 1), (2, 3), (4, 5), (6, 7), (8, 9), (10, 11), (12,)]
    for h in range(HLOC):
        dh_sb = sb["dn"].tile([HD + 1, NCHUNK, NQ], F32R, tag="dn", name="dn")
        for c in range(NCHUNK):
            oe_ps = ps["oe"].tile([HD + 1, NQ], F32, tag="oe", name="oe")

            def emit_scores(pair):
                sc_ps = ps["sc"].tile([P, 2, 512], F32, tag="sc", name="sc")
                for j, kt in enumerate(pair):
                    nc.tensor.matmul(
                        sc_ps[:, j, :NQ], _r(k_sb[:, h, kt * P:(kt + 1) * P]),
                        _r(_chunk(q_sb[:, h // 2], c)), start=True, stop=True)
                return sc_ps

            def emit_exp_av(sc_ps, pair):
                p_sb = sb["p"].tile([P, 2, NQ], BF16, tag="p", name="p")
                npair = len(pair)
                nc.scalar.activation(
                    p_sb[:, :npair], sc_ps[:, :npair, :NQ], AF.Exp,
                    scale=float(HD) ** -0.5)
                for j, kt in enumerate(pair):
                    nc.tensor.matmul(
                        oe_ps[:], _r(v_sb[:, kt, h]), _r(p_sb[:, j]),
                        start=(kt == 0), stop=(kt == NKT - 1))

            # scores run one kt-pair ahead of exp+AV so the PE never waits
            # on the ScalarE exp of the pair it just produced
            pending = None
            for pair in KT_PAIRS:
                sc_ps = emit_scores(pair)
                if pending is not None:
                    emit_exp_av(*pending)
                pending = (sc_ps, pair)
            emit_exp_av(*pending)
            nc.vector.tensor_copy(dh_sb[:, c], oe_ps[:])
        # batch-normalize: one in-place reciprocal over all 4 chunk rowsums
        with nc.allow_low_precision(reason="softmax denom recip"):
            nc.vector.reciprocal(dh_sb[HD:HD + 1], dh_sb[HD:HD + 1])
        for c in range(NCHUNK):
            rb_ps = ps["mm"].tile([HD, NQ], F32, tag="mm", name="rb")
            nc.tensor.matmul(rb_ps[:], _r(ones_sb[HD:HD + 1, 0:HD]),
                             _r(dh_sb[HD:HD + 1, c]),
                             start=True, stop=True)
            ds_sb = sb["ds"].tile([HD, NQ], F32, tag="ds", name="ds")
            nc.vector.tensor_mul(ds_sb[:], dh_sb[0:HD, c], rb_ps[:])
            nc.sync.dma_start(
                cc_in_s[HD * h:HD * (h + 1), c * NQ:(c + 1) * NQ], ds_sb[:])


def _stage_stream(nc, ps, sb, ins, i, s, t_sb, tb_sb, kpad_sb, cc_in_s, cons):
    src_q = t_sb[s]
    src_kv = tb_sb[1 - s]
    src_qb = tb_sb[s]

    wq_sb = sb["w"].tile([P, CSUB, 256], BF16, tag="wq", name="wq")
    wk_sb = sb["w"].tile([P, CSUB, 256], BF16, tag="wk", name="wk")
    wv_sb = sb["w"].tile([P, CSUB, 256], BF16, tag="wv", name="wv")
    nc.sync.dma_start(wq_sb[:], ins["wq"][i, s].rearrange("(su p) m -> p su m", p=P))
    nc.sync.dma_start(wk_sb[:], ins["wk"][i, s].rearrange("(su p) m -> p su m", p=P))
    nc.sync.dma_start(wv_sb[:], ins["wv"][i, s].rearrange("(su p) m -> p su m", p=P))
    g_row = sb["cst"].tile([1, CSUB, P], F32R, tag="lng", name="lng")
    nc.sync.dma_start(g_row[:],
                      ins["lng"][i, s].rearrange("(su p) -> su p", p=P)[None])
    beta_col = sb["cst"].tile([P, CSUB], F32, tag="lnb", name="lnb")
    nc.sync.dma_start(beta_col[:], ins["lnb"][i, s].rearrange("(su p) -> p su", p=P))
    bq_sb = sb["cst"].tile([P, 2], F32, tag="bq", name="bq")
    bk_sb = sb["cst"].tile([P, 2], F32, tag="bk", name="bk")
    nc.sync.dma_start(bq_sb[:], ins["bq"][i, s].rearrange("(m p) -> p m", p=P))
    nc.sync.dma_start(bk_sb[:], ins["bk"][i, s].rearrange("(m p) -> p m", p=P))

    ln_sb = _layernorm(nc, ps, sb, src_q, src_qb, g_row, beta_col, cons)

    q_sb = sb["qk"].tile([P, 2, N], BF16, tag="q", name="q")
    k_sb = kpad_sb
    _proj_cm(nc, ps, ln_sb, wq_sb, bq_sb, q_sb)
    _proj_k(nc, ps, src_kv, wk_sb, bk_sb, k_sb)
    v_sb = _proj_v(nc, ps, sb, src_kv, wv_sb, cons[0])

    _attention(nc, ps, sb, q_sb, k_sb, v_sb, cc_in_s, cons[0])


def _residual_update(nc, sb, ins, i, s, t_sb, cc_out):  # cc_out: [2,256,N]
    bv_sb = sb["cst"].tile([P, CSUB], F32, tag="bv", name="bv")
    nc.sync.dma_start(bv_sb[:], ins["bv"][i, s].rearrange("(su p) -> p su", p=P))
    for r in range(2):
        g_sb = sb["big"].tile([P, 2, N], F32, tag="big", name="gather")
        nc.sync.dma_start(g_sb[:], cc_out[r].rearrange("(m p) n -> p m n", p=P))
        for m in range(2):
            su = 2 * r + m
            nc.vector.scalar_tensor_tensor(
                t_sb[s][:, su], g_sb[:, m], bv_sb[:, su:su + 1],
                t_sb[s][:, su], ALU.add, ALU.add)


def _out_head(nc, ps, sb, ins, s, t_sb, out):
    wo_sb = sb["w"].tile([P, CSUB, 256], F32R, tag="wo", name="wo")
    nc.sync.dma_start(wo_sb[:], ins["wo"][s].rearrange("(su p) m -> p su m", p=P))
    ob_sb = sb["cst"].tile([P, 2], F32, tag="ob", name="ob")
    bns_sb = sb["cst"].tile([P, 2], F32, tag="bns", name="bns")
    bnb_sb = sb["cst"].tile([P, 2], F32, tag="bnb", name="bnb")
    nc.sync.dma_start(ob_sb[:], ins["ob"][s].rearrange("(m p) -> p m", p=P))
    nc.sync.dma_start(bns_sb[:], ins["bns"][s].rearrange("(m p) -> p m", p=P))
    nc.sync.dma_start(bnb_sb[:], ins["bnb"][s].rearrange("(m p) -> p m", p=P))
    o_sb = sb["qk"].tile([P, 2, N], F32, tag="q", name="ohead")
    for m in range(2):
        for c in range(NCHUNK):
            z_ps = ps["mm"].tile([P, NQ], F32, tag="mm", name="zps")
            for su in range(CSUB):
                nc.tensor.matmul(
                    z_ps[:], _r(wo_sb[:, su, m * P:(m + 1) * P]),
                    _r(_chunk(t_sb[s][:, su], c)),
                    start=(su == 0), stop=(su == CSUB - 1))
            dst = _chunk(o_sb[:, m], c)
            nc.scalar.activation(dst, z_ps[:], AF.Relu, bias=ob_sb[:, m:m + 1])
            nc.vector.tensor_scalar(
                dst, dst, bns_sb[:, m:m + 1], bnb_sb[:, m:m + 1], ALU.mult, ALU.add)
    nc.sync.dma_start(out[s].rearrange("(m p) n -> p m n", p=P), o_sb[:])


def build_nc():
    nc = bacc.Bacc("TRN2", target_bir_lowering=False, debug=False,
                   num_devices=N_CORES)

    ins = {}
    ins["x"] = nc.dram_tensor("x", [2, C, N], F32R, kind="ExternalInput")
    ins["ones"] = nc.dram_tensor("ones", [P, P], F32R, kind="ExternalInput")
    ins["wq"] = nc.dram_tensor("wq", [STAGES, 2, C, 256], BF16, kind="ExternalInput")
    ins["wk"] = nc.dram_tensor("wk", [STAGES, 2, C, 256], BF16, kind="ExternalInput")
    ins["wv"] = nc.dram_tensor("wv", [STAGES, 2, C, 256], BF16, kind="ExternalInput")
    ins["bq"] = nc.dram_tensor("bq", [STAGES, 2, 256], F32, kind="ExternalInput")
    ins["bk"] = nc.dram_tensor("bk", [STAGES, 2, 256], F32, kind="ExternalInput")
    ins["bv"] = nc.dram_tensor("bv", [STAGES, 2, C], F32, kind="ExternalInput")
    ins["lng"] = nc.dram_tensor("lng", [STAGES, 2, C], F32R, kind="ExternalInput")
    ins["lnb"] = nc.dram_tensor("lnb", [STAGES, 2, C], F32, kind="ExternalInput")
    ins["wo"] = nc.dram_tensor("wo", [2, C, 256], F32R, kind="ExternalInput")
    ins["ob"] = nc.dram_tensor("ob", [2, 256], F32, kind="ExternalInput")
    ins["bns"] = nc.dram_tensor("bns", [2, 256], F32, kind="ExternalInput")
    ins["bnb"] = nc.dram_tensor("bnb", [2, 256], F32, kind="ExternalInput")
    out = nc.dram_tensor("out", [2, 256, N], F32, kind="ExternalOutput")

    with ExitStack() as ctx:
        tc = ctx.enter_context(tile.TileContext(nc))
        sb = {
            "res": ctx.enter_context(tc.tile_pool(name="res", bufs=1)),
            "big": ctx.enter_context(tc.tile_pool(name="big", bufs=1)),
            "bigb": ctx.enter_context(tc.tile_pool(name="bigb", bufs=2)),
            "tb": ctx.enter_context(tc.tile_pool(name="tb", bufs=1)),
            "qk": ctx.enter_context(tc.tile_pool(name="qk", bufs=1)),
            "v": ctx.enter_context(tc.tile_pool(name="vv", bufs=1)),
            "w": ctx.enter_context(tc.tile_pool(name="wt", bufs=1)),
            "sm": ctx.enter_context(tc.tile_pool(name="sm", bufs=1)),
            "cst": ctx.enter_context(tc.tile_pool(name="cst", bufs=2)),
            "p": ctx.enter_context(tc.tile_pool(name="pp", bufs=4)),
            "dn": ctx.enter_context(tc.tile_pool(name="dn", bufs=2)),
            "ds": ctx.enter_context(tc.tile_pool(name="dsp", bufs=4)),
        }
        ps = {
            "mm": ctx.enter_context(tc.tile_pool(name="ps_mm", bufs=2, space="PSUM")),
            "sc": ctx.enter_context(tc.tile_pool(name="ps_sc", bufs=2, space="PSUM")),
            "oe": ctx.enter_context(tc.tile_pool(name="ps_oe", bufs=2, space="PSUM")),
        }
        dram = ctx.enter_context(tc.tile_pool(name="dram", bufs=1, space="DRAM"))

        t_sb = [sb["res"].tile([P, CSUB, N], F32R, tag=f"t{s}", name=f"t{s}")
                for s in (0, 1)]
        kpad_sb = sb["res"].tile([P, HLOC, NPAD], BF16, tag="kpad", name="kpad")
        ones_sb = sb["res"].tile([P, P], F32R, tag="ones", name="ones")
        nc.sync.dma_start(ones_sb[:], ins["ones"][:])
        eps_sb = sb["res"].tile([1, 1], F32, tag="eps", name="eps")
        nc.vector.memset(eps_sb[:], EPS)
        ones_bf = sb["res"].tile([P, 1], BF16, tag="onesbf", name="onesbf")
        nc.vector.tensor_copy(ones_bf[:], ones_sb[:, 0:1])
        cons = (ones_sb, eps_sb, ones_bf)
        for s in (0, 1):
            nc.sync.dma_start(t_sb[s][:],
                              ins["x"][s].rearrange("(su p) n -> p su n", p=P))
        # zero the pad halves (even-head slots: rows 64:128, odd: 0:64) and
        # the padded key-token columns; written once, preserved across stages
        zsrc = ones_sb[:, 0:1]
        nc.vector.tensor_scalar_mul(
            kpad_sb[HD:P, 0:HLOC:2],
            zsrc[HD:P].to_broadcast((HD, 2, NPAD)), 0.0)
        nc.vector.tensor_scalar_mul(
            kpad_sb[0:HD, 1:HLOC:2],
            zsrc[0:HD].to_broadcast((HD, 2, NPAD)), 0.0)
        nc.vector.tensor_scalar_mul(
            kpad_sb[0:HD, 0:HLOC:2, N:NPAD],
            zsrc[0:HD].to_broadcast((HD, 2, NPAD - N)), 0.0)
        nc.vector.tensor_scalar_mul(
            kpad_sb[HD:P, 1:HLOC:2, N:NPAD],
            zsrc[HD:P].to_broadcast((HD, 2, NPAD - N)), 0.0)

        for i in range(STAGES):
            tb_sb = [sb["tb"].tile([P, CSUB, N], BF16, tag=f"tb{s}", name=f"tb{s}")
                     for s in (0, 1)]
            for s in (0, 1):
                nc.vector.tensor_copy(tb_sb[s][:], t_sb[s][:])
            cc_outs = []
            for s in (0, 1):
                cc_in = dram.tile([256, N], F32, name="cc_in")
                cc_out = dram.tile([2, 256, N], F32, name="cc_out")
                cc_outs.append(cc_out)
                _stage_stream(nc, ps, sb, ins, i, s, t_sb, tb_sb, kpad_sb, cc_in,
                              cons)
                nc.gpsimd.collective_compute(
                    "AllGather", ALU.bypass, replica_groups=REPLICA_GROUPS,
                    ins=[cc_in[:]], outs=[cc_out[:]])
            for s in (0, 1):
                _residual_update(nc, sb, ins, i, s, t_sb, cc_outs[s])

        for s in (0, 1):
            _out_head(nc, ps, sb, ins, s, t_sb, out)

    nc.compile()
    return nc


_NC_CACHE = {}


def _get_nc():
    if "nc" not in _NC_CACHE:
        _NC_CACHE["nc"] = build_nc()
    return _NC_CACHE["nc"]


def _prep_in_maps(inputs):
    import ml_dtypes
    f = lambda k: np.ascontiguousarray(np.asarray(inputs[k], np.float32))
    bf = lambda a: np.ascontiguousarray(a.astype(ml_dtypes.bfloat16))
    x1, x2 = f("x1"), f("x2")
    bn_fold = np.float32(1.0 / np.sqrt(np.float32(1.0) + np.float32(EPS)))

    per_g = []
    for g in range(2):
        gs = slice(256 * g, 256 * (g + 1))
        d = {
            "wq": bf(np.stack([np.stack([f("Wq1")[i].T[:, gs], f("Wq2")[i].T[:, gs]])
                            for i in range(STAGES)])),
            "wk": bf(np.stack([np.stack([f("Wk1")[i].T[:, gs], f("Wk2")[i].T[:, gs]])
                            for i in range(STAGES)])),
            "wv": bf(np.stack([np.stack([f("Wv1")[i].T[:, gs], f("Wv2")[i].T[:, gs]])
                            for i in range(STAGES)])),
            "bq": np.stack([np.stack([f("bq1")[i][gs], f("bq2")[i][gs]])
                            for i in range(STAGES)]),
            "bk": np.stack([np.stack([f("bk1")[i][gs], f("bk2")[i][gs]])
                            for i in range(STAGES)]),
            "bv": np.stack([np.stack([f("bv1")[i], f("bv2")[i]])
                            for i in range(STAGES)]),
            "lng": np.stack([np.stack([f("ln1_g")[i], f("ln2_g")[i]])
                             for i in range(STAGES)]),
            "lnb": np.stack([np.stack([f("ln1_b")[i], f("ln2_b")[i]])
                             for i in range(STAGES)]),
            "wo": np.stack([f("out1_w").T[:, gs], f("out2_w").T[:, gs]]),
            "ob": np.stack([f("out1_b")[gs], f("out2_b")[gs]]),
            "bns": np.stack([f("bn1_g")[gs] * bn_fold, f("bn2_g")[gs] * bn_fold]),
            "bnb": np.stack([f("bn1_b")[gs], f("bn2_b")[gs]]),
        }
        per_g.append({k: np.ascontiguousarray(v) for k, v in d.items()})

    in_maps = []
    for core in range(N_CORES):
        b, g = core // 2, core % 2
        m = dict(per_g[g])
        m["x"] = np.ascontiguousarray(
            np.stack([x1[b].reshape(C, N), x2[b].reshape(C, N)]))
        m["ones"] = np.ones((P, P), np.float32)
        in_maps.append(m)
    return in_maps


def kernel(**inputs):
    nc = _get_nc()
    in_maps = _prep_in_maps(inputs)
    res = run_bass_kernel_spmd(nc, in_maps, core_ids=list(range(N_CORES)))
    z1 = np.empty((4, C, 8, 14, 14), np.float32)
    z2 = np.empty((4, C, 32, 7, 7), np.float32)
    for core in range(N_CORES):
        b, g = core // 2, core % 2
        o = res.results[core]["out"]
        z1[b, 256 * g:256 * (g + 1)] = o[0].reshape(256, 8, 14, 14)
        z2[b, 256 * g:256 * (g + 1)] = o[1].reshape(256, 32, 7, 7)
    return (z1, z2)
